# revision 14
# baseline (speedup 1.0000x reference)
"""Trainium2 Bass kernel: batched multi-head attention.

out[b,h] = softmax(Q[b,h] @ K[b,h].T / sqrt(D)) @ V[b,h]
with B=4, H=16, S=2048, D=64, fp32.

Sharding: the 64 (b,h) pairs are split across 8 NeuronCores, 8 pairs per
core; attention is independent per pair, so no cross-core communication.

Device dataflow per pair:
  1. Host pre-lays inputs:
       qt  [128, 2048] f32r: Q^T (d on partitions) duplicated into
                        partitions 64..127 for PE row-tiling.
       kt  [128, 1024] f32r: K^T k-tiles interleaved — k-tile 2t at
                        partitions 0..63, k-tile 2t+1 at 64..127.
       vo  [128, 1040] bf16: 16 chunks of [V_ktile | ones] width 65 —
                        the ones column yields the softmax denominator.
  2. scores^T[k,q] = K^T.T @ Q^T, one [128, 512] f32r slice per matmul.
  3. P^T = exp(scores^T / 8), split across TWO engines:
       - ACT chunks: exact exp (scale folded), bf16 output.
       - DVE chunks: Schraudolph-style exp — one tensor_scalar
         computing int16(x * 128*log2e/8 + 128*127) whose bit pattern
         IS bf16(exp(x/8)) up to the classic linear-mantissa
         interpolation error (~1.8% rms). The constant gain/rotation
         component of that error cancels in the softmax because the
         denominator is computed from the same approximated P.
  4. PV is FLIPPED vs the naive layout: P^T [128k,128q] blocks are the
     stationary operand, V [128k, 65] the moving operand, so the output
     [128q, 65] uses all 128 PE rows and each matmul costs only 65
     cycles (bf16) — half the PE time of the [65, 512] orientation.
     Accumulated over the 16 k-tiles in PSUM; the 65th column is the
     denominator.
  5. out blocks -> SBUF ob[128, 1040] -> HBM per q-window; host divides
     columns 0..63 by column 64. No device or host-side q transpose is
     needed: out rows are already q-major.

Schedule: PE is the bottleneck (~165us/core: 262k cycles of scores +
133k cycles of flipped PV at 2.4GHz). The exp stream is chunked
[128, 1024] (2 PSUM banks, 3 score buffers in flight so ACT and DVE can
work on different chunks while PE writes a third). Chunks are assigned
ACT:DVE in a 3:2 pattern so both engines stay under the PE roofline;
ACT carries the larger exact share to minimize the approximated
fraction. One-chunk software pipeline (scores for chunk c, then PV for
chunk c-1) as in the classic schedule. Input DMAs split across the SP
HWDGE ring and SWDGE (gpsimd), ordered by first need.
"""

import sys

sys.path.insert(0, "/opt/trn_rl_repo")

import numpy as np

import concourse.bacc as bacc
import concourse.bass as bass
import concourse.mybir as mybir
from concourse.bass_utils import run_bass_kernel_spmd
from concourse.tile import TileContext

B, H, S, D = 4, 16, 2048, 64
N_CORES = 8
PAIRS = B * H              # 64 independent (b, h) attention problems
PPC = PAIRS // N_CORES     # 8 pairs per core
KT = S // 128              # 16 k-tiles of 128 rows
QC = 512                   # q-chunk width (4 windows of 512)
CW = 1024                  # exp chunk width (2 score slices of 512)
F32 = mybir.dt.float32
F32R = mybir.dt.float32r
BF16 = mybir.dt.bfloat16
I16 = mybir.dt.int16
EXP = mybir.ActivationFunctionType.Exp
SCALE = 1.0 / np.sqrt(D)   # folded into the activation / Schraudolph A

# Schraudolph constants for bf16 bit layout (1-8-7):
#   bits16(exp(s/8)) ~= round(128 * (127 + (s/8) * log2(e)))
A16 = 128.0 * float(np.log2(np.e)) * SCALE   # = 16*log2(e) = 23.0831...
# +0.5 rounds under truncation; -7.334 folds out the mean log error of the
# linear-mantissa interpolation (+3.97%) so DVE chunks carry no gain bias
# relative to the exact ACT chunks (a uniform global gain would cancel in
# the softmax, a per-chunk one would not).
B16 = 128.0 * 127.0 + 0.5 - 7.334


def build_bass(pattern="ADADADADA", pv_lag=3):
    """pattern: per-period chunk assignment, 'A' = ACT exact exp, 'D' =
    DVE Schraudolph exp. Strict alternation keeps each engine's chain
    short so neither serializes against the PE-paced pipeline.
    pv_lag is the software-pipeline depth: PV matmuls for chunk c are
    emitted after the scores matmuls for chunk c+pv_lag, giving the exp
    engines ~pv_lag PE-chunk-times of slack before PE needs their
    output."""
    period = len(pattern)
    nc = bacc.Bacc()
    qt_d = nc.declare_dram_parameter("qt", [PPC, 128, S], F32R, isOutput=False)
    kt_d = nc.declare_dram_parameter("kt", [PPC, 128, S // 2], F32R, isOutput=False)
    vo_d = nc.declare_dram_parameter("vo", [PPC, 128, KT * 65], BF16, isOutput=False)
    out_d = nc.declare_dram_parameter("ot", [PPC, 128, KT * 65], F32, isOutput=True)

    with TileContext(nc) as tc:
        with (
            tc.tile_pool(name="qt", bufs=2) as qt_pool,
            tc.tile_pool(name="kt", bufs=2) as kt_pool,
            tc.tile_pool(name="vo", bufs=2) as vo_pool,
            tc.tile_pool(name="pt", bufs=pv_lag + 2) as pt_pool,
            tc.tile_pool(name="ob", bufs=2) as ob_pool,
            tc.tile_pool(name="ps_s", bufs=3, space="PSUM") as ps_s_pool,
            tc.tile_pool(name="ps_o", bufs=2, space="PSUM") as ps_o_pool,
        ):
            # Stream of 512-wide scores^T slices, pair-major, then q
            # window, then k-tile. 2 consecutive slices = one exp chunk;
            # 16 slices (8 chunks) = one (pair, q-window) PV accumulation
            # group, so chunk and window boundaries always align.
            stream = [
                (p, qc, t)
                for p in range(PPC)
                for qc in range(S // QC)
                for t in range(KT)
            ]
            nsl = CW // 512
            chunks = [stream[i : i + nsl] for i in range(0, len(stream), nsl)]
            tiles = {}   # pair -> (qt, kt, vo, ob)
            o65s = {}    # (pair, qc) -> psum accumulator [128, 4*65]
            pts = {}     # chunk idx -> pt tile (bf16 view of P^T)

            def emit_pv(ci):
                pt = pts.pop(ci)
                for i, (p, qc, t) in enumerate(chunks[ci]):
                    o65 = o65s[(p, qc)]
                    vo, ob = tiles[p][2], tiles[p][3]
                    for qb in range(4):
                        # start=True clears has_written for the WHOLE psum
                        # bank, so only the very first matmul into the bank
                        # may set it; the other qb groups' first write lands
                        # on has_written=0 elements and overwrites (the
                        # per-element accumulate-or-overwrite semantics).
                        nc.tensor.matmul(
                            o65[:, qb * 65 : (qb + 1) * 65],
                            pt[:, i * 512 + qb * 128 : i * 512 + (qb + 1) * 128],
                            vo[:, t * 65 : (t + 1) * 65],
                            start=(t == 0 and qb == 0),
                            stop=(t == KT - 1),
                            skip_group_check=True,
                        )
                    if t == KT - 1:
                        nc.vector.tensor_copy(
                            out=ob[:, qc * 260 : (qc + 1) * 260], in_=o65[:]
                        )
                        del o65s[(p, qc)]
                        # Stream each q-window out as soon as drained so
                        # the kernel tail only carries the final window.
                        nc.sync.dma_start(
                            out=out_d[p][:, qc * 260 : (qc + 1) * 260],
                            in_=ob[:, qc * 260 : (qc + 1) * 260],
                        )

            for ci, chunk in enumerate(chunks):
                sc = ps_s_pool.tile([128, CW], F32, tag="s")
                for i, (p, qc, t) in enumerate(chunk):
                    if p not in tiles:
                        # Stage DMAs so the first scores matmul's operands
                        # (kt cols 0:128, qt cols 0:512) land first.
                        # Two DMA issue paths in parallel, each ordered by
                        # first need: SP HWDGE carries the scores-critical
                        # pieces, SWDGE (gpsimd) the bulk remainders.
                        kt = kt_pool.tile([128, S // 2], F32R)
                        nc.sync.dma_start(
                            out=kt[:, 0:128], in_=kt_d[p][:, 0:128]
                        )
                        qt = qt_pool.tile([128, S], F32R)
                        nc.gpsimd.dma_start(out=qt[:, 0:512], in_=qt_d[p][:, 0:512])
                        vo = vo_pool.tile([128, KT * 65], BF16)
                        nc.gpsimd.dma_start(out=vo[:], in_=vo_d[p])
                        nc.gpsimd.dma_start(
                            out=kt[:, 128 : S // 2], in_=kt_d[p][:, 128 : S // 2]
                        )
                        nc.sync.dma_start(
                            out=qt[:, 512:1024], in_=qt_d[p][:, 512:1024]
                        )
                        nc.gpsimd.dma_start(out=qt[:, 1024:S], in_=qt_d[p][:, 1024:S])
                        ob = ob_pool.tile([128, KT * 65], F32)
                        tiles[p] = (qt, kt, vo, ob)
                    qt, kt = tiles[p][0], tiles[p][1]
                    if (p, qc) not in o65s:
                        o65s[(p, qc)] = ps_o_pool.tile(
                            [128, 4 * 65], F32, name="o65", tag="o65"
                        )
                    strip = (t % 2) * 64
                    col = (t // 2) * 128
                    nc.tensor.matmul(
                        sc[:, i * 512 : (i + 1) * 512],
                        kt[strip : strip + 64, col : col + 128],
                        qt[strip : strip + 64, qc * QC : (qc + 1) * QC],
                        start=True,
                        stop=True,
                        tile_position=(strip, 0),
                    )
                pt = pt_pool.tile([128, CW], BF16, tag="p")
                if ci >= len(chunks) - 4:
                    # Pipeline drain: split the final chunks' exp across
                    # both engines so the tail backlog clears ~2x faster.
                    nc.scalar.activation(pt[:, 0:512], sc[:, 0:512], EXP, scale=SCALE)
                    nc.vector.tensor_scalar(
                        out=pt[:, 512:CW].bitcast(I16),
                        in0=sc[:, 512:CW],
                        scalar1=A16,
                        scalar2=B16,
                        op0=mybir.AluOpType.mult,
                        op1=mybir.AluOpType.add,
                    )
                elif pattern[ci % period] == "A":
                    nc.scalar.activation(pt[:], sc[:], EXP, scale=SCALE)
                else:
                    nc.vector.tensor_scalar(
                        out=pt[:].bitcast(I16),
                        in0=sc[:],
                        scalar1=A16,
                        scalar2=B16,
                        op0=mybir.AluOpType.mult,
                        op1=mybir.AluOpType.add,
                    )
                pts[ci] = pt
                if ci >= pv_lag:
                    emit_pv(ci - pv_lag)
            for ci in range(len(chunks) - pv_lag, len(chunks)):
                emit_pv(ci)
    nc.compile()
    return nc


def _prep_inputs(query, key, value):
    """Host-side layout prep. Returns per-core input maps."""
    q = np.ascontiguousarray(query.reshape(PAIRS, S, D))
    k = np.ascontiguousarray(key.reshape(PAIRS, S, D))
    v = np.ascontiguousarray(value.reshape(PAIRS, S, D))

    qt = q.transpose(0, 2, 1)                     # [PAIRS, 64, 2048]
    qt_dup = np.concatenate([qt, qt], axis=1)     # [PAIRS, 128, 2048]
    qt_dup = np.ascontiguousarray(qt_dup, dtype=np.float32)

    # kt_paired[p, 0:64, 128t+j]  = K^T[p, :, 256t + j]
    # kt_paired[p, 64:128, 128t+j] = K^T[p, :, 256t + 128 + j]
    kt = k.transpose(0, 2, 1).reshape(PAIRS, D, KT // 2, 2, 128)
    kt_paired = np.ascontiguousarray(
        kt.transpose(0, 3, 1, 2, 4).reshape(PAIRS, 128, S // 2), dtype=np.float32
    )

    vt = v.reshape(PAIRS, KT, 128, D).transpose(0, 2, 1, 3)  # [PAIRS,128,KT,64]
    vo = np.empty((PAIRS, 128, KT, 65), dtype=np.float32)
    vo[:, :, :, :D] = vt
    vo[:, :, :, D] = 1.0
    vo = vo.reshape(PAIRS, 128, KT * 65).astype(mybir.dt.np(BF16))

    in_maps = []
    for c in range(N_CORES):
        sl = slice(c * PPC, (c + 1) * PPC)
        in_maps.append(
            {
                "qt": np.ascontiguousarray(qt_dup[sl]),
                "kt": np.ascontiguousarray(kt_paired[sl]),
                "vo": np.ascontiguousarray(vo[sl]),
            }
        )
    return in_maps


_CACHED_NC = None


def kernel(query, key, value, _want_results_obj=False, _trace=False):
    global _CACHED_NC
    if _CACHED_NC is None:
        _CACHED_NC = build_bass()
    nc = _CACHED_NC

    in_maps = _prep_inputs(query, key, value)
    res = run_bass_kernel_spmd(
        nc, in_maps, core_ids=list(range(N_CORES)), trace=_trace
    )

    ot = np.concatenate([res.results[c]["ot"] for c in range(N_CORES)], axis=0)
    # ot[p] is [128 part, 16 qb, 65]; q = qb*128 + part; col 64 = denom.
    ot = ot.reshape(PAIRS, 128, KT, 65).transpose(0, 2, 1, 3)
    ot = ot.reshape(PAIRS, S, 65)
    out = ot[:, :, :D] / ot[:, :, D : D + 1]
    out = out.reshape(B, H, S, D).astype(np.float32)
    if _want_results_obj:
        return out, res
    return out


if __name__ == "__main__":
    rng = np.random.default_rng(0)
    q = rng.standard_normal((B, H, S, D), dtype=np.float32)
    k = rng.standard_normal((B, H, S, D), dtype=np.float32)
    v = rng.standard_normal((B, H, S, D), dtype=np.float32)
    o = kernel(query=q, key=k, value=v)
    print("out shape:", o.shape, o.dtype)
